# revision 1
# baseline (speedup 1.0000x reference)
"""Trainium2 Bass kernel for nn_CrossAttention (B=8, C=256, W=H=64).

Sharding: data-parallel over batch across the 8 NeuronCores; the small 1x1
conv weights and gamma are replicated.

Per-core computation (one batch, xf = x reshaped [C, N], N = W*H = 4096):
    f   = Wf @ xf            [CQ, N]   (bf16 matmul, f32 psum)
    g   = Wg @ yf            [CQ, N]
    hhT = (Wh @ xf + bh).T   [N, C]    (stored transposed, + ones column)
    LT[j, i]  = sum_d g[d, j] f[d, i]          (transposed logits)
    E = exp(LT)  (no max subtraction: logits are O(60) max, exp fits f32/bf16)
    UT[i, c] = sum_j E[j, i] * hhT[j, c]   -> col C holds D[i] = sum_j E[j, i]
    satT[i, c] = UT[i, c] * gamma / D[i]
    out[c, i] = satT.T + x[c, i]
"""

import numpy as np

import concourse.bass as bass
import concourse.mybir as mybir
import concourse.tile as tile
from concourse import bacc
from concourse.bass import ds, ts
from concourse.bass_utils import run_bass_kernel_spmd
from concourse.masks import make_identity

FP32 = mybir.dt.float32
BF16 = mybir.dt.bfloat16
AF = mybir.ActivationFunctionType
ALU = mybir.AluOpType

C = 256
CQ = 32
N_CORES = 8


def build_nc(n=4096, jq=None):
    """Build the single-core Bass program (SPMD across cores via inputs)."""
    assert n % 128 == 0
    NB = n // 128            # number of 128-row j blocks
    IC = min(512, n)         # i-chunk size for the main loop
    NIC = n // IC            # number of i chunks
    NQ = IC // 128           # 128-row i tiles per chunk
    FCH = min(512, n)        # free-dim chunk for the f/g matmuls
    JQ = jq if jq else 4     # j blocks per PE row-tiling group
    assert NB % JQ == 0
    # each packed matmul must land in its own PSUM bank (512 f32 per bank)
    assert IC == 512

    nc = bacc.Bacc("TRN2", target_bir_lowering=False, debug=False)

    x_d = nc.dram_tensor("x", [C, n], FP32, kind="ExternalInput").ap()
    y_d = nc.dram_tensor("y", [C, n], FP32, kind="ExternalInput").ap()
    wfT_d = nc.dram_tensor("wfT", [C, CQ], FP32, kind="ExternalInput").ap()
    wgT_d = nc.dram_tensor("wgT", [C, CQ], FP32, kind="ExternalInput").ap()
    whT_d = nc.dram_tensor("whT", [C, C], FP32, kind="ExternalInput").ap()
    bf_d = nc.dram_tensor("bf", [CQ, 1], FP32, kind="ExternalInput").ap()
    bg_d = nc.dram_tensor("bg", [CQ, 1], FP32, kind="ExternalInput").ap()
    bh_d = nc.dram_tensor("bh", [1, C], FP32, kind="ExternalInput").ap()
    gamma_d = nc.dram_tensor("gamma", [1, 1], FP32, kind="ExternalInput").ap()
    out_d = nc.dram_tensor("out", [C, n], FP32, kind="ExternalOutput").ap()

    with tile.TileContext(nc) as tc:
        with tc.tile_pool(name="persist", bufs=1) as persist, \
             tc.tile_pool(name="consts", bufs=1) as consts:
            # ---- persistent SBUF tensors -------------------------------
            x_sb = persist.tile([128, 2, n], FP32, tag="x_sb")
            y_sb = persist.tile([128, 2, n], FP32, tag="y_sb")
            xb = persist.tile([128, 2, n], BF16, tag="xb")
            yb = persist.tile([128, 2, n], BF16, tag="yb")
            f_rep = persist.tile([128, n], BF16, tag="f_rep")   # 4 replicated bands
            g_rep = persist.tile([128, n], BF16, tag="g_rep")
            hhT = persist.tile([128, NB, C + 1], BF16, tag="hhT")

            wfT_f = consts.tile([128, 2, CQ], FP32, tag="wfT_f")
            wgT_f = consts.tile([128, 2, CQ], FP32, tag="wgT_f")
            whT_f = consts.tile([128, 2, C], FP32, tag="whT_f")
            wfT_b = consts.tile([128, 2, CQ], BF16, tag="wfT_b")
            wgT_b = consts.tile([128, 2, CQ], BF16, tag="wgT_b")
            whT_b = consts.tile([128, 2, C], BF16, tag="whT_b")
            bf_sb = consts.tile([CQ, 1], FP32, tag="bf_sb")
            bg_sb = consts.tile([CQ, 1], FP32, tag="bg_sb")
            bh_f = consts.tile([1, C], FP32, tag="bh_f")
            bh_b = consts.tile([1, C], BF16, tag="bh_b")
            ones_b = consts.tile([1, 128], BF16, tag="ones_b")
            gamma_sb = consts.tile([128, 1], FP32, tag="gamma_sb")
            ident = consts.tile([128, 128], BF16, tag="ident")

            # ---- weight DMAs + casts (small, first) --------------------
            for cb in range(2):
                nc.sync.dma_start(out=wfT_f[:, cb, :], in_=wfT_d[cb * 128:(cb + 1) * 128, :])
                nc.sync.dma_start(out=wgT_f[:, cb, :], in_=wgT_d[cb * 128:(cb + 1) * 128, :])
                nc.sync.dma_start(out=whT_f[:, cb, :], in_=whT_d[cb * 128:(cb + 1) * 128, :])
            nc.sync.dma_start(out=bf_sb, in_=bf_d[:, :])
            nc.sync.dma_start(out=bg_sb, in_=bg_d[:, :])
            nc.sync.dma_start(out=bh_f, in_=bh_d[:, :])
            nc.sync.dma_start(out=gamma_sb, in_=gamma_d[:, :].to_broadcast([128, 1]))
            for cb in range(2):
                nc.vector.tensor_copy(wfT_b[:, cb, :], wfT_f[:, cb, :])
                nc.vector.tensor_copy(wgT_b[:, cb, :], wgT_f[:, cb, :])
                nc.vector.tensor_copy(whT_b[:, cb, :], whT_f[:, cb, :])
            nc.vector.tensor_copy(bh_b, bh_f)
            nc.vector.memset(ones_b, 1.0)
            make_identity(nc, ident)

            # ---- chunked input DMAs + casts (overlap with compute) -----
            DCH = min(1024, n)
            for ch in range(n // DCH):
                for cb in range(2):
                    nc.sync.dma_start(out=x_sb[:, cb, ts(ch, DCH)],
                                      in_=x_d[cb * 128:(cb + 1) * 128, ts(ch, DCH)])
                    nc.sync.dma_start(out=y_sb[:, cb, ts(ch, DCH)],
                                      in_=y_d[cb * 128:(cb + 1) * 128, ts(ch, DCH)])
                for cb in range(2):
                    nc.vector.tensor_copy(xb[:, cb, ts(ch, DCH)], x_sb[:, cb, ts(ch, DCH)])
                    nc.vector.tensor_copy(yb[:, cb, ts(ch, DCH)], y_sb[:, cb, ts(ch, DCH)])

            # ---- phase A: f, g, hhT ------------------------------------
            with tc.tile_pool(name="psA", bufs=4, space="PSUM") as psA:
                NCH = n // FCH
                for ch in range(NCH):
                    pf = psA.tile([CQ, FCH], FP32, tag="pA")
                    for cb in range(2):
                        nc.tensor.matmul(pf, lhsT=wfT_b[:, cb, :], rhs=xb[:, cb, ts(ch, FCH)],
                                         start=(cb == 0), stop=(cb == 1))
                    nc.vector.tensor_scalar_add(f_rep[0:CQ, ts(ch, FCH)], pf, bf_sb)
                    pg = psA.tile([CQ, FCH], FP32, tag="pA")
                    for cb in range(2):
                        nc.tensor.matmul(pg, lhsT=wgT_b[:, cb, :], rhs=yb[:, cb, ts(ch, FCH)],
                                         start=(cb == 0), stop=(cb == 1))
                    nc.vector.tensor_scalar_add(g_rep[0:CQ, ts(ch, FCH)], pg, bg_sb)
                    # band replication per finished half (overlaps with next DMAs)
                    if ch == NCH - 1:
                        for r in range(1, 4):
                            nc.sync.dma_start(out=f_rep[32 * r:32 * (r + 1), :],
                                              in_=f_rep[0:32, :])
                            nc.sync.dma_start(out=g_rep[32 * r:32 * (r + 1), :],
                                              in_=g_rep[0:32, :])
                # hhT = (Wh @ x + bh).T, stored [j, c] with ones in col C
                for jb in range(NB):
                    ph = psA.tile([128, C], FP32, tag="pA")
                    nc.tensor.matmul(ph, lhsT=xb[:, 0, ts(jb, 128)], rhs=whT_b[:, 0, :],
                                     start=True, stop=False)
                    nc.tensor.matmul(ph, lhsT=xb[:, 1, ts(jb, 128)], rhs=whT_b[:, 1, :],
                                     start=False, stop=False)
                    nc.tensor.matmul(ph, lhsT=ones_b, rhs=bh_b,
                                     start=False, stop=True)
                    nc.vector.tensor_copy(hhT[:, jb, 0:C], ph)
                    nc.vector.memset(hhT[:, jb, C:C + 1], 1.0)

            # ---- main attention loop -----------------------------------
            # PSUM banks: lt 4 bufs x 1 bank + ut/tp 4 bufs x 1 = 8.
            with tc.tile_pool(name="ut", bufs=4, space="PSUM") as utp, \
                 tc.tile_pool(name="lt", bufs=4, space="PSUM") as ltp, \
                 tc.tile_pool(name="ex", bufs=8) as exp_pool, \
                 tc.tile_pool(name="tail", bufs=8) as tailp, \
                 tc.tile_pool(name="stage", bufs=3) as stagep:
                for ic in range(NIC):
                    uts = [utp.tile([128, C + 1], FP32, tag="ut", name=f"ut{q}") for q in range(NQ)]
                    for jg in range(NB // JQ):
                        # JQ row-group-packed logit matmuls, each into its own bank
                        lts = [ltp.tile([128, IC], FP32, tag="lt", name=f"lt{jj}")
                               for jj in range(JQ)]
                        for jj in range(JQ):
                            j = jg * JQ + jj
                            nc.tensor.matmul(
                                lts[jj],
                                lhsT=g_rep[32 * jj:32 * (jj + 1), ts(j, 128)],
                                rhs=f_rep[32 * jj:32 * (jj + 1), ds(ic * IC, IC)],
                                start=True, stop=True,
                                tile_position=(32 * jj, 0))
                        exs = []
                        for jj in range(JQ):
                            ex = exp_pool.tile([128, IC], BF16, tag="ex", name=f"ex{jj}")
                            nc.scalar.activation(ex, lts[jj], AF.Exp)
                            exs.append(ex)
                        # group UT matmuls by psum bank: JQ back-to-back MMs per
                        # bank avoids the per-MM psum-queue switch penalty
                        for q in range(NQ):
                            for jj in range(JQ):
                                j = jg * JQ + jj
                                nc.tensor.matmul(
                                    uts[q],
                                    lhsT=exs[jj][:, ds(q * 128, 128)],
                                    rhs=hhT[:, j, :],
                                    start=(j == jg * JQ and jg == 0),
                                    stop=(j == jg * JQ + JQ - 1 and jg == NB // JQ - 1),
                                    skip_group_check=True)
                    # tail: normalize, transpose back, residual add, store
                    stage = stagep.tile([128, 2, IC], FP32, tag="stage")
                    for q in range(NQ):
                        i0 = ic * IC + q * 128
                        rd = tailp.tile([128, 1], FP32, tag="rd")
                        nc.vector.reciprocal(rd, uts[q][:, C:C + 1])
                        satT = tailp.tile([128, C], BF16, tag="satT")
                        nc.vector.tensor_scalar(satT, uts[q][:, 0:C], rd, gamma_sb,
                                                op0=ALU.mult, op1=ALU.mult)
                        for cb in range(2):
                            tp = utp.tile([128, 128], BF16, tag="ut", name="tp")
                            nc.tensor.transpose(tp, satT[:, ds(cb * 128, 128)], ident)
                            nc.vector.tensor_add(stage[:, cb, ds(q * 128, 128)],
                                                 tp, x_sb[:, cb, ds(i0, 128)])
                    for cb in range(2):
                        nc.sync.dma_start(out=out_d[cb * 128:(cb + 1) * 128, ds(ic * IC, IC)],
                                          in_=stage[:, cb, :])

    nc.compile()
    return nc


_NC_CACHE = {}


def _get_nc(n=4096):
    if n not in _NC_CACHE:
        _NC_CACHE[n] = build_nc(n)
    return _NC_CACHE[n]


def make_in_maps(x, y, Wf, bf, Wg, bg, Wh, bh, gamma):
    x = np.asarray(x, dtype=np.float32)
    y = np.asarray(y, dtype=np.float32)
    B, C_, W_, H_ = x.shape
    n = W_ * H_
    wfT = np.ascontiguousarray(np.asarray(Wf, np.float32).T)
    wgT = np.ascontiguousarray(np.asarray(Wg, np.float32).T)
    whT = np.ascontiguousarray(np.asarray(Wh, np.float32).T)
    bf_ = np.asarray(bf, np.float32).reshape(CQ, 1)
    bg_ = np.asarray(bg, np.float32).reshape(CQ, 1)
    bh_ = np.asarray(bh, np.float32).reshape(1, C_)
    gm_ = np.asarray(gamma, np.float32).reshape(1, 1)
    in_maps = []
    for b in range(B):
        in_maps.append({
            "x": np.ascontiguousarray(x[b].reshape(C_, n)),
            "y": np.ascontiguousarray(y[b].reshape(C_, n)),
            "wfT": wfT, "wgT": wgT, "whT": whT,
            "bf": bf_, "bg": bg_, "bh": bh_, "gamma": gm_,
        })
    return in_maps, (B, C_, W_, H_)


def run_spmd(inputs: dict, trace: bool = False):
    """Run the SPMD kernel; returns (out [B,C,W,H], BassKernelResults)."""
    in_maps, (B, C_, W_, H_) = make_in_maps(**inputs)
    nc = _get_nc(W_ * H_)
    res = run_bass_kernel_spmd(nc, in_maps, core_ids=list(range(B)), trace=trace)
    out = np.stack([res.results[b]["out"].reshape(C_, W_, H_) for b in range(B)])
    return np.ascontiguousarray(out, dtype=np.float32), res


def kernel(x, y, Wf, bf, Wg, bg, Wh, bh, gamma):
    out, _ = run_spmd(dict(x=x, y=y, Wf=Wf, bf=bf, Wg=Wg, bg=bg,
                           Wh=Wh, bh=bh, gamma=gamma))
    return out

